# revision 1
# baseline (speedup 1.0000x reference)
"""Trainium2 Bass kernel for nn_AutoCorrelation (8 NeuronCores, data-parallel over batch).

Algorithm (reference: AutoCorrelation block):
  corr = irfft(rfft(q, L) * conj(rfft(k, L)))        # circular cross-correlation
  top-6 delays from batch-mean of corr (mean over H,E then N)
  out  = sum_k softmax(mean[:, idx])_k * roll(v, -idx_k)

Implementation:
  - FFTs become dense DFT matmuls on the TensorEngine: rfft -> q @ C and
    q @ Sm with C[l,f]=cos(2*pi*l*f/L), Sm[l,f]=-sin(...), f=0..511, and the
    Nyquist (f=512) cosine column packed into Sm[:,0] (sin column there is 0).
    irfft -> Pre @ A + Pim @ B with the matching inverse rows (A[0]=DC row,
    B[0]=Nyquist row).
  - Phase 1 kernel (per core, 4 batch items): forward DFTs, pointwise complex
    product (VectorE), inverse DFT, corr written to HBM, per-delay row-sums
    reduced for the top-k statistic.
  - Host: tiny (N,L) mean -> top-6 indices + softmax weights.
  - Phase 2 kernel: out = sum_k w*roll(v) as PSUM-accumulated matmuls with
    w-scaled shifted-identity stationary matrices (shift along L = partition
    permutation, contracted on the TensorEngine).
"""
import math
import sys

sys.path.insert(0, "/opt/trn_rl_repo")

import numpy as np
import ml_dtypes

import concourse.bass as bass
import concourse.tile as tile
from concourse import bacc, mybir
from concourse.bass import ts
from concourse.bass_utils import run_bass_kernel_spmd

_dt = mybir.dt

N, L, H, E = 32, 1024, 8, 64
R = H * E                 # 512 rows (h,e) per batch item
NCORES = 8
NLOC = N // NCORES        # 4 batch items per core
F = 512                   # packed rfft freqs (f=0..511; Nyquist in slot 0)
F2 = 256                  # freqs per radix-2 half (even / odd)
TOPK = int(1.0 * math.log(L))  # 6
LB = L // 128             # 8 l/tau blocks
FB = F // 128             # 4 f blocks
HB = 4                    # 128-blocks per 512-half

# phase-1 matmul dtype: "f32r" (full fp32 precision at ~bf16 rate) or "bf16"
P1_MODE = "bf16"
TRACE = [False]           # test.py flips this to collect exec_time_ns
LAST_EXEC_NS = [0, 0]     # phase1, phase2 exec time (when TRACE)


def _dft_mats():
    """Radix-2 split matrices. Forward (contract over l' = 0..511):
    even freqs X[2m] = (x1+x2) @ [C5 | S5m] (S5m slot 0 = f=512 Nyquist),
    odd freqs X[2m+1] = (x1-x2) @ [Mre | Mim] (twiddle folded in).
    Inverse: u = Pe_re@Au + Pe_im@Bu, w = Po_re@Aw + Po_im@Bw,
    corr[t] = u+w, corr[t+512] = u-w."""
    l = np.arange(512)[:, None].astype(np.float64)
    m = np.arange(F2)[None, :].astype(np.float64)
    C5 = np.cos(2 * np.pi * l * m / 512)
    S5 = -np.sin(2 * np.pi * l * m / 512)
    S5[:, 0] = (-1.0) ** np.arange(512)
    Mre = np.cos(2 * np.pi * l * (2 * m + 1) / L)
    Mim = -np.sin(2 * np.pi * l * (2 * m + 1) / L)
    t = np.arange(512)[None, :].astype(np.float64)
    mm = np.arange(F2)[:, None].astype(np.float64)
    Au = (2.0 / L) * np.cos(2 * np.pi * mm * t / 512)
    Bu = -(2.0 / L) * np.sin(2 * np.pi * mm * t / 512)
    Au[0, :] = 1.0 / L
    Bu[0, :] = (1.0 / L) * ((-1.0) ** np.arange(512))
    Aw = (2.0 / L) * np.cos(2 * np.pi * t * (2 * mm + 1) / L)
    Bw = -(2.0 / L) * np.sin(2 * np.pi * t * (2 * mm + 1) / L)
    return C5, S5, Mre, Mim, Au, Bu, Aw, Bw


def _build_phase1(mode):
    store = _dt.bfloat16

    nc = bacc.Bacc("TRN2", target_bir_lowering=False, debug=False,
                   num_devices=NCORES)
    q_d = nc.dram_tensor("q", [NLOC, L, R], store, kind="ExternalInput").ap()
    k_d = nc.dram_tensor("k", [NLOC, L, R], store, kind="ExternalInput").ap()
    cst_d = {}
    for nm in ("c5", "s5", "mre", "mim"):
        cst_d[nm] = nc.dram_tensor(nm, [512, F2], store,
                                   kind="ExternalInput").ap()
    for nm in ("au", "bu", "aw", "bw"):
        cst_d[nm] = nc.dram_tensor(nm, [F2, 512], store,
                                   kind="ExternalInput").ap()
    corr_d = nc.dram_tensor("corr", [NLOC, L, R], _dt.bfloat16,
                            kind="ExternalOutput").ap()
    # per-group row-sums of P: cols 0..3 = re (E0,E1,O0,O1), 4..7 = im
    pacc_d = nc.dram_tensor("pacc", [NLOC, 128, 8], _dt.float32,
                            kind="ExternalOutput").ap()

    def mm(ps, lhsT, rhs, start, stop):
        nc.tensor.matmul(ps, lhsT, rhs, start=start, stop=stop)

    with tile.TileContext(nc) as tc:
        with tc.tile_pool(name="const", bufs=1) as cp, \
             tc.tile_pool(name="qk", bufs=20) as qk, \
             tc.tile_pool(name="ed", bufs=12) as edp, \
             tc.tile_pool(name="pp", bufs=8) as pp, \
             tc.tile_pool(name="tmp", bufs=3) as tp, \
             tc.tile_pool(name="out", bufs=6) as op, \
             tc.tile_pool(name="ps", bufs=5, space="PSUM") as psf, \
             tc.tile_pool(name="psi", bufs=3, space="PSUM") as psi:

            # Head-latency-ordered loads, DMA issue spread over sync+scalar.
            # First chains need c5 + q (then k, s5; odd/inverse mats later).
            cmats = {}
            q0, k0 = [], []
            for j in range(HB):
                t = cp.tile([128, F2], store, tag=f"c5{j}")
                nc.sync.dma_start(t[:], cst_d["c5"][ts(j, 128), :])
                cmats.setdefault("c5", []).append(t)
            # (j, j+4) pair order so butterfly j can start after 2 tiles
            q0, k0 = [None] * LB, [None] * LB
            for i, lb in enumerate((0, 4, 1, 5, 2, 6, 3, 7)):
                t = qk.tile([128, R], store, tag="q")
                (nc.scalar if i % 2 else nc.sync).dma_start(
                    t[:], q_d[0, ts(lb, 128), :])
                q0[lb] = t
            for i, lb in enumerate((0, 4, 1, 5, 2, 6, 3, 7)):
                t = qk.tile([128, R], store, tag="k")
                (nc.scalar if i % 2 else nc.sync).dma_start(
                    t[:], k_d[0, ts(lb, 128), :])
                k0[lb] = t
            for j in range(HB):
                t = cp.tile([128, F2], store, tag=f"s5{j}")
                nc.sync.dma_start(t[:], cst_d["s5"][ts(j, 128), :])
                cmats.setdefault("s5", []).append(t)
            for nm in ("mre", "mim"):
                for j in range(HB):
                    t = cp.tile([128, F2], store, tag=f"{nm}{j}")
                    nc.scalar.dma_start(t[:], cst_d[nm][ts(j, 128), :])
                    cmats.setdefault(nm, []).append(t)
            for nm in ("au", "bu", "aw", "bw"):
                for j in range(2):
                    t = cp.tile([128, 512], store, tag=f"{nm}{j}")
                    nc.sync.dma_start(t[:], cst_d[nm][ts(j, 128), :])
                    cmats.setdefault(nm, []).append(t)

            for n in range(NLOC):
                if n == 0:
                    q_sb, k_sb = q0, k0
                else:
                    q_sb, k_sb = [None] * LB, [None] * LB
                    for i, lb in enumerate((0, 4, 1, 5, 2, 6, 3, 7)):
                        t = qk.tile([128, R], store, tag="q")
                        nc.sync.dma_start(t[:], q_d[n, ts(lb, 128), :])
                        q_sb[lb] = t
                        t = qk.tile([128, R], store, tag="k")
                        nc.scalar.dma_start(t[:], k_d[n, ts(lb, 128), :])
                        k_sb[lb] = t

                # radix-2 butterflies, each split column-wise GpSimd/DVE so
                # neither engine's op latency paces the forward chains
                eq, dq, ek, dk = [], [], [], []
                HR = R // 2
                for tag, lst, x_sb, fn in (("eq", eq, q_sb, "tensor_add"),
                                           ("dq", dq, q_sb, "tensor_sub"),
                                           ("ek", ek, k_sb, "tensor_add"),
                                           ("dk", dk, k_sb, "tensor_sub")):
                    for j in range(HB):
                        t = edp.tile([128, R], store, tag=tag)
                        getattr(nc.gpsimd, fn)(
                            t[:, 0:HR], x_sb[j][:, 0:HR], x_sb[j + 4][:, 0:HR])
                        getattr(nc.vector, fn)(
                            t[:, HR:R], x_sb[j][:, HR:R], x_sb[j + 4][:, HR:R])
                        lst.append(t)

                acc = op.tile([128, 8], _dt.float32, tag="acc")
                pre_sb, pim_sb = [], []
                groups = [("c5", "s5", eq, ek, 0), ("c5", "s5", eq, ek, 1),
                          ("mre", "mim", dq, dk, 0), ("mre", "mim", dq, dk, 1)]
                for gi, (ma, mb_, xq, xk, mb) in enumerate(groups):
                    MA, MB = cmats[ma], cmats[mb_]
                    ps_qre = psf.tile([128, R], _dt.float32, tag="fwd")
                    ps_qim = psf.tile([128, R], _dt.float32, tag="fwd")
                    ps_kre = psf.tile([128, R], _dt.float32, tag="fwd")
                    ps_kim = psf.tile([128, R], _dt.float32, tag="fwd")
                    for j in range(HB):
                        mm(ps_qre[:], MA[j][:, ts(mb, 128)], xq[j][:],
                           j == 0, j == HB - 1)
                    for j in range(HB):
                        mm(ps_kre[:], MA[j][:, ts(mb, 128)], xk[j][:],
                           j == 0, j == HB - 1)
                    for j in range(HB):
                        mm(ps_qim[:], MB[j][:, ts(mb, 128)], xq[j][:],
                           j == 0, j == HB - 1)
                    for j in range(HB):
                        mm(ps_kim[:], MB[j][:, ts(mb, 128)], xk[j][:],
                           j == 0, j == HB - 1)

                    # stage Q/K to bf16 SBUF (DVE 2x mode for the muls);
                    # copies split DVE/ACT to balance engine load
                    qre = tp.tile([128, R], store, tag="qre")
                    qim = tp.tile([128, R], store, tag="qim")
                    kre = tp.tile([128, R], store, tag="kre")
                    kim = tp.tile([128, R], store, tag="kim")
                    nc.scalar.mul(qre[:], ps_qre[:], 1.0)
                    nc.scalar.mul(qim[:], ps_qim[:], 1.0)
                    nc.scalar.mul(kre[:], ps_kre[:], 1.0)
                    nc.scalar.mul(kim[:], ps_kim[:], 1.0)
                    t1 = tp.tile([128, R], store, tag="t1")
                    t2 = tp.tile([128, R], store, tag="t2")
                    nc.vector.tensor_mul(t1[:], qre[:], kre[:])
                    nc.vector.tensor_mul(t2[:], qim[:], kim[:])
                    pre = pp.tile([128, R], store, tag="pre")
                    nc.vector.scalar_tensor_tensor(
                        pre[:], t1[:], 1.0, t2[:],
                        op0=mybir.AluOpType.mult, op1=mybir.AluOpType.add,
                        accum_out=acc[:, gi:gi + 1])
                    t3 = tp.tile([128, R], store, tag="t3")
                    t4 = tp.tile([128, R], store, tag="t4")
                    nc.vector.tensor_mul(t3[:], qim[:], kre[:])
                    nc.vector.tensor_mul(t4[:], qre[:], kim[:])
                    pim = pp.tile([128, R], store, tag="pim")
                    nc.vector.scalar_tensor_tensor(
                        pim[:], t3[:], 1.0, t4[:],
                        op0=mybir.AluOpType.mult, op1=mybir.AluOpType.subtract,
                        accum_out=acc[:, 4 + gi:5 + gi])
                    if gi == 0:
                        # slot 0 packs DC (re) / Nyquist (im): overwrite with
                        # pure products and patch the two accum elements
                        nc.vector.tensor_copy(pre[0:1, :], t1[0:1, :])
                        nc.vector.tensor_copy(pim[0:1, :], t2[0:1, :])
                        nc.vector.tensor_reduce(
                            acc[0:1, 0:1], t1[0:1, :],
                            axis=mybir.AxisListType.X, op=mybir.AluOpType.add)
                        nc.vector.tensor_reduce(
                            acc[0:1, 4:5], t2[0:1, :],
                            axis=mybir.AxisListType.X, op=mybir.AluOpType.add)
                    pre_sb.append(pre)
                    pim_sb.append(pim)

                for tb in range(HB):
                    ps_u = psi.tile([128, R], _dt.float32, tag="inv")
                    ps_w = psi.tile([128, R], _dt.float32, tag="inv")
                    for gb in range(2):
                        mm(ps_u[:], cmats["au"][gb][:, ts(tb, 128)],
                           pre_sb[gb][:], gb == 0, False)
                        mm(ps_u[:], cmats["bu"][gb][:, ts(tb, 128)],
                           pim_sb[gb][:], False, gb == 1)
                    for gb in range(2):
                        mm(ps_w[:], cmats["aw"][gb][:, ts(tb, 128)],
                           pre_sb[2 + gb][:], gb == 0, False)
                        mm(ps_w[:], cmats["bw"][gb][:, ts(tb, 128)],
                           pim_sb[2 + gb][:], False, gb == 1)
                    w_sb = tp.tile([128, R], _dt.float32, tag="wsb")
                    nc.scalar.mul(w_sb[:], ps_w[:], 1.0)
                    corr_lo = op.tile([128, R], store, tag="clo")
                    corr_hi = op.tile([128, R], store, tag="chi")
                    nc.vector.tensor_add(corr_lo[:], ps_u[:], w_sb[:])
                    nc.vector.tensor_sub(corr_hi[:], ps_u[:], w_sb[:])
                    nc.sync.dma_start(corr_d[n, ts(tb, 128), :], corr_lo[:])
                    nc.scalar.dma_start(corr_d[n, ts(tb + HB, 128), :],
                                        corr_hi[:])
                nc.sync.dma_start(pacc_d[n][:], acc[:])
    nc.compile()
    return nc

def _build_phase2(entries):
    """entries: per output block b, list of (src_block, seg_idx); seg_idx
    indexes the g stationaries tensor (NLOC, NSEG, 128, 128)."""
    nseg = max(si for segs in entries for _, si in segs) + 1
    nc = bacc.Bacc("TRN2", target_bir_lowering=False, debug=False,
                   num_devices=NCORES)
    v_d = nc.dram_tensor("v", [NLOC, L, R], _dt.bfloat16,
                         kind="ExternalInput").ap()
    # g is host-packed as (NLOC, 128, nseg*128): one contiguous DMA per n;
    # stationary si is the [:, si*128:(si+1)*128] slice.
    g_d = nc.dram_tensor("g", [NLOC, 128, nseg * 128], _dt.bfloat16,
                         kind="ExternalInput").ap()
    out_d = nc.dram_tensor("out", [NLOC, L, R], _dt.bfloat16,
                           kind="ExternalOutput").ap()

    with tile.TileContext(nc) as tc:
        with tc.tile_pool(name="v", bufs=16) as vp, \
             tc.tile_pool(name="g", bufs=NLOC) as gp, \
             tc.tile_pool(name="o", bufs=6) as op, \
             tc.tile_pool(name="ps", bufs=8, space="PSUM") as psp:
            # v[0] first (first matmul dep), then the stationaries (tiny),
            # then the remaining v prefetch as compute proceeds.
            g_sb = []
            v0 = []
            for a in range(LB):
                t = vp.tile([128, R], _dt.bfloat16, tag="v")
                (nc.scalar if a % 2 else nc.sync).dma_start(
                    t[:], v_d[0, ts(a, 128), :])
                v0.append(t)
                if a == 1:
                    tg = gp.tile([128, nseg * 128], _dt.bfloat16, tag="g")
                    nc.sync.dma_start(tg[:], g_d[0][:])
                    g_sb.append(tg)
            for n in range(1, NLOC):
                t = gp.tile([128, nseg * 128], _dt.bfloat16, tag="g")
                nc.scalar.dma_start(t[:], g_d[n][:])
                g_sb.append(t)
            for n in range(NLOC):
                if n == 0:
                    v_sb = v0
                else:
                    v_sb = []
                    for a in range(LB):
                        t = vp.tile([128, R], _dt.bfloat16, tag="v")
                        (nc.scalar if a % 2 else nc.sync).dma_start(
                            t[:], v_d[n, ts(a, 128), :])
                        v_sb.append(t)
                for b in range(LB):
                    segs = entries[b]
                    ps = psp.tile([128, R], _dt.float32, tag="ps")
                    for i, (a, si) in enumerate(segs):
                        nc.tensor.matmul(ps[:], g_sb[n][:, ts(si, 128)],
                                         v_sb[a][:],
                                         start=(i == 0),
                                         stop=(i == len(segs) - 1))
                    o_sb = op.tile([128, R], _dt.bfloat16, tag="o")
                    nc.vector.tensor_copy(o_sb[:], ps[:])
                    (nc.scalar if b % 2 else nc.sync).dma_start(
                        out_d[n, ts(b, 128), :], o_sb[:])
    nc.compile()
    return nc


_P1_CACHE = {}


def _phase1_nc(mode):
    if mode not in _P1_CACHE:
        _P1_CACHE[mode] = _build_phase1(mode)
    return _P1_CACHE[mode]


def _run(nc, in_maps, phase):
    res = run_bass_kernel_spmd(nc, in_maps, core_ids=list(range(NCORES)),
                               trace=TRACE[0])
    if TRACE[0]:
        LAST_EXEC_NS[phase] = res.exec_time_ns
    return res.results


def kernel(queries, keys, values):
    queries = np.ascontiguousarray(np.asarray(queries, dtype=np.float32))
    keys = np.ascontiguousarray(np.asarray(keys, dtype=np.float32))
    values = np.ascontiguousarray(np.asarray(values, dtype=np.float32))

    mode = P1_MODE
    store_np = ml_dtypes.bfloat16
    C5, S5, Mre, Mim, Au, Bu, Aw, Bw = _dft_mats()
    consts = {
        "c5": C5, "s5": S5, "mre": Mre, "mim": Mim,
        "au": Au, "bu": Bu, "aw": Aw, "bw": Bw,
    }
    consts = {k: np.ascontiguousarray(v.astype(np.float32)).astype(store_np)
              for k, v in consts.items()}

    q3 = queries.reshape(N, L, R)
    k3 = keys.reshape(N, L, R)
    v3 = values.reshape(N, L, R)

    nc1 = _phase1_nc(mode)
    in_maps = []
    for c in range(NCORES):
        sl = slice(c * NLOC, (c + 1) * NLOC)
        in_maps.append({
            "q": q3[sl].astype(store_np),
            "k": k3[sl].astype(store_np),
            **consts,
        })
    res1 = _run(nc1, in_maps, 0)

    corr = np.concatenate([r["corr"] for r in res1], axis=0)  # (N, L, R) f32
    pacc = np.concatenate([r["pacc"] for r in res1], axis=0)  # (N, 128, 8)
    # reconstruct mean over (H,E) from per-group P row-sums (host irfft on
    # a 512-vector per batch item)
    pacc = pacc.astype(np.float64)
    per_ = pacc[:, :, 0:2].transpose(0, 2, 1).reshape(N, 256)   # Pe_re sums
    por_ = pacc[:, :, 2:4].transpose(0, 2, 1).reshape(N, 256)   # Po_re
    pei_ = pacc[:, :, 4:6].transpose(0, 2, 1).reshape(N, 256)   # Pe_im
    poi_ = pacc[:, :, 6:8].transpose(0, 2, 1).reshape(N, 256)   # Po_im
    um = per_ @ Au + pei_ @ Bu
    wm = por_ @ Aw + poi_ @ Bw
    mean = np.concatenate([um + wm, um - wm], axis=1) / R       # (N, L)

    g = mean.mean(axis=0)
    idx = np.argsort(-g, kind="stable")[:TOPK]
    w = mean[:, idx]
    e = np.exp(w - w.max(axis=1, keepdims=True))
    w = (e / e.sum(axis=1, keepdims=True)).astype(np.float32)  # (N, TOPK)

    # phase-2 stationaries: out[b*128+j] += w_k * v[(b*128+j+idx_k) mod L]
    # merged per (b, src_block); matrix content is b-independent, so dedup
    # identical segment sets across b.
    seg_of = {}
    pat = []
    entries = [[] for _ in range(LB)]
    for b in range(LB):
        acc = {}
        for kk in range(TOPK):
            sh = int(idx[kk])
            r = sh % 128
            a = ((b * 128 + sh) // 128) % LB
            acc.setdefault(a, []).append(("d1", r, kk))
            if r > 0:
                acc.setdefault((a + 1) % LB, []).append(("d2", r, kk))
        for a, parts in sorted(acc.items()):
            key = tuple(sorted(parts))
            if key not in seg_of:
                seg_of[key] = len(pat)
                pat.append(parts)
            entries[b].append((a, seg_of[key]))
    nseg = len(pat)
    gmat = np.zeros((NLOC * NCORES, nseg, 128, 128), np.float32)
    jj = np.arange(128)
    for si, parts in enumerate(pat):
        for which, r, kk in parts:
            if which == "d1":
                j = jj[: 128 - r]
                gmat[:, si, j + r, j] += w[:, kk][:, None]
            else:
                j = jj[128 - r:]
                gmat[:, si, j - (128 - r), j] += w[:, kk][:, None]
    # pack (NLOC, nseg, 128, 128) -> (NLOC, 128, nseg*128) for 1-DMA-per-n
    gmat = np.ascontiguousarray(
        gmat.transpose(0, 2, 1, 3).reshape(NLOC * NCORES, 128, nseg * 128)
    ).astype(ml_dtypes.bfloat16)

    nc2 = _build_phase2(entries)
    in_maps2 = []
    for c in range(NCORES):
        sl = slice(c * NLOC, (c + 1) * NLOC)
        in_maps2.append({
            "v": v3[sl].astype(ml_dtypes.bfloat16),
            "g": gmat[sl],
        })
    res2 = _run(nc2, in_maps2, 1)
    out = np.concatenate([np.asarray(r["out"], dtype=np.float32)
                          for r in res2], axis=0)             # (N, L, R)

    out_full = out.reshape(N, L, H, E).astype(np.float32)
    corr_full = corr.reshape(N, L, H, E).astype(np.float32)
    return out_full, corr_full



# revision 7
# speedup vs baseline: 1.0331x; 1.0331x over previous
"""Trainium2 Bass kernel for nn_AutoCorrelation (8 NeuronCores, data-parallel over batch).

Algorithm (reference: AutoCorrelation block):
  corr = irfft(rfft(q, L) * conj(rfft(k, L)))        # circular cross-correlation
  top-6 delays from batch-mean of corr (mean over H,E then N)
  out  = sum_k softmax(mean[:, idx])_k * roll(v, -idx_k)

Implementation (two launches, host does only the tiny (N,L) topk/softmax glue):
  - Phase 1: radix-2-real DFT as dense TensorE matmuls. Engine balance per
    batch item targets the PE pace (~20.5us): PE 96 matmuls, ACT stages all
    PSUM->SBUF bf16 conversions as double-bank [128,1024] copies, DVE does
    the complex products (tensor_tensor_reduce carries the topk row-sum
    stat) + final u+/-w combines, GpSimd a slice of the radix-2 butterflies.
    All HBM traffic is mega-tile DMAs (1 per tensor per batch item) to kill
    descriptor-dispatch overhead and keep the PE continuously fed so it
    holds the 2.4 GHz p-state.
  - Phase 2: out = sum_k w*roll(v) as PSUM-accumulated matmuls with w-scaled
    shifted-identity stationaries (built on host after the free host topk).
"""
import math
import sys

sys.path.insert(0, "/opt/trn_rl_repo")

import numpy as np
import ml_dtypes

import concourse.bass as bass
import concourse.tile as tile
from concourse import bacc, mybir
from concourse.bass import ts
from concourse.bass_utils import run_bass_kernel_spmd

_dt = mybir.dt

N, L, H, E = 32, 1024, 8, 64
R = H * E                 # 512 rows (h,e) per batch item
NCORES = 8
NLOC = N // NCORES        # 4 batch items per core
F2 = 256                  # freqs per radix-2 half (even / odd)
TOPK = int(1.0 * math.log(L))  # 6
LB = L // 128             # 8 l/tau blocks
HB = 4                    # 128-blocks per 512-half

TRACE = [False]           # test.py flips this to collect exec_time_ns
LAST_EXEC_NS = [0, 0]     # phase1, phase2 exec time (when TRACE)

# HW-bisect knobs
P1_GPSIMD_DMA = [True]    # issue some DMAs from the gpsimd queue (SWDGE)
P1_FUSED_PSUM = [True]    # [128,1024] two-bank PSUM tiles
P1_MEGA_DMA = [True]      # 3-D mega-tile DMAs (1 per tensor per batch)


def _dft_mats():
    """Radix-2 split matrices. Forward (contract over l' = 0..511):
    even freqs X[2m] = (x1+x2) @ [C5 | S5m] (S5m slot 0 = f=512 Nyquist),
    odd freqs X[2m+1] = (x1-x2) @ [Mre | Mim] (twiddle folded in).
    Inverse: u = Pe_re@Au + Pe_im@Bu, w = Po_re@Aw + Po_im@Bw,
    corr[t] = u+w, corr[t+512] = u-w."""
    l = np.arange(512)[:, None].astype(np.float64)
    m = np.arange(F2)[None, :].astype(np.float64)
    C5 = np.cos(2 * np.pi * l * m / 512)
    S5 = -np.sin(2 * np.pi * l * m / 512)
    S5[:, 0] = (-1.0) ** np.arange(512)
    Mre = np.cos(2 * np.pi * l * (2 * m + 1) / L)
    Mim = -np.sin(2 * np.pi * l * (2 * m + 1) / L)
    t = np.arange(512)[None, :].astype(np.float64)
    mm = np.arange(F2)[:, None].astype(np.float64)
    Au = (2.0 / L) * np.cos(2 * np.pi * mm * t / 512)
    Bu = -(2.0 / L) * np.sin(2 * np.pi * mm * t / 512)
    Au[0, :] = 1.0 / L
    Bu[0, :] = (1.0 / L) * ((-1.0) ** np.arange(512))
    Aw = (2.0 / L) * np.cos(2 * np.pi * t * (2 * mm + 1) / L)
    Bw = -(2.0 / L) * np.sin(2 * np.pi * t * (2 * mm + 1) / L)
    return C5, S5, Mre, Mim, Au, Bu, Aw, Bw


def _build_phase1():
    store = _dt.bfloat16

    nc = bacc.Bacc("TRN2", target_bir_lowering=False, debug=False,
                   num_devices=NCORES)
    q_d = nc.dram_tensor("q", [NLOC, L, R], store, kind="ExternalInput").ap()
    k_d = nc.dram_tensor("k", [NLOC, L, R], store, kind="ExternalInput").ap()
    # cf cols: c5 | s5 | mre | mim  (each [512, 256])
    cf_d = nc.dram_tensor("cf", [512, 4 * F2], store,
                          kind="ExternalInput").ap()
    # ci cols: au | bu | aw | bw  (each [256, 512])
    ci_d = nc.dram_tensor("ci", [F2, 4 * 512], store,
                          kind="ExternalInput").ap()
    corr_d = nc.dram_tensor("corr", [NLOC, L, R], store,
                            kind="ExternalOutput").ap()
    # per-group P row-sums: cols 0..3 = re (E0,E1,O0,O1), 4..7 = im
    pacc_d = nc.dram_tensor("pacc", [NLOC, 128, 8], _dt.float32,
                            kind="ExternalOutput").ap()

    def mm(ps, lhsT, rhs, start, stop):
        nc.tensor.matmul(ps, lhsT, rhs, start=start, stop=stop)

    with tile.TileContext(nc) as tc:
        with tc.tile_pool(name="const", bufs=1) as cp, \
             tc.tile_pool(name="qk", bufs=2) as qk, \
             tc.tile_pool(name="ed", bufs=8) as edp, \
             tc.tile_pool(name="st", bufs=6) as stp, \
             tc.tile_pool(name="tmp", bufs=3) as tp, \
             tc.tile_pool(name="pp", bufs=10) as pp, \
             tc.tile_pool(name="uw", bufs=4) as uwp, \
             tc.tile_pool(name="out", bufs=2) as op, \
             tc.tile_pool(name="psq", bufs=1, space="PSUM") as psq, \
             tc.tile_pool(name="psk", bufs=1, space="PSUM") as psk, \
             tc.tile_pool(name="psi", bufs=2, space="PSUM") as psi:

            # pipeline fill over the 3 dma-capable queues (sync/scalar/
            # gpsimd): q0 split for latency, cf right behind it
            qm_all, km_all = [], []
            t = qk.tile([128, LB, R], store, tag="q")
            nc.sync.dma_start(t[:, 0:HB, :],
                              q_d[0, 0:512].rearrange("(j p) r -> p j r",
                                                      p=128))
            nc.scalar.dma_start(t[:, HB:LB, :],
                                q_d[0, 512:1024].rearrange(
                                    "(j p) r -> p j r", p=128))
            qm_all.append(t)
            t = qk.tile([128, LB, R], store, tag="k")
            nc.gpsimd.dma_start(t[:], k_d[0].rearrange("(j p) r -> p j r",
                                                       p=128))
            km_all.append(t)
            cf_sb = cp.tile([128, HB, 4 * F2], store, tag="cf")
            nc.sync.dma_start(cf_sb[:],
                              cf_d.rearrange("(j p) c -> p j c", p=128))
            ci_sb = cp.tile([128, 2, 4 * 512], store, tag="ci")
            nc.scalar.dma_start(ci_sb[:],
                                ci_d.rearrange("(g p) c -> p g c", p=128))
            t = qk.tile([128, LB, R], store, tag="q")
            nc.gpsimd.dma_start(t[:], q_d[1].rearrange("(j p) r -> p j r",
                                                       p=128))
            qm_all.append(t)
            t = qk.tile([128, LB, R], store, tag="k")
            nc.scalar.dma_start(t[:], k_d[1].rearrange("(j p) r -> p j r",
                                                       p=128))
            km_all.append(t)

            def fwd_st(name_i, j, mb):
                off = name_i * F2 + mb * 128
                return cf_sb[:, j, off:off + 128]

            def inv_st(name_i, gb, tb):
                off = name_i * 512 + tb * 128
                return ci_sb[:, gb, off:off + 128]

            state = [None] * NLOC  # per-n (pre_sb, pim_sb, acc) for inverse

            def forward(n):
                qm, km = qm_all[n], km_all[n]
                # butterflies: full-width ops, DVE 10 / GpSimd 6 (GpSimd
                # gets the late-consumed dq/dk tiles)
                eq, dq, ek, dk = [], [], [], []
                for tag, lst, x, fn in (("eq", eq, qm, "tensor_add"),
                                        ("dq", dq, qm, "tensor_sub"),
                                        ("ek", ek, km, "tensor_add"),
                                        ("dk", dk, km, "tensor_sub")):
                    for j in range(HB):
                        t = edp.tile([128, R], store, tag=tag)
                        slow = tag in ("dq", "dk") and j >= 1
                        eng = nc.gpsimd if slow else nc.vector
                        getattr(eng, fn)(t[:], x[:, j, :], x[:, j + 4, :])
                        lst.append(t)

                acc = op.tile([128, 8], _dt.float32, tag="acc")
                pre_sb, pim_sb = [], []
                groups = [(0, 1, eq, ek, 0), (0, 1, eq, ek, 1),
                          (2, 3, dq, dk, 0), (2, 3, dq, dk, 1)]
                for gi, (ma, mb_, xq, xk, mb) in enumerate(groups):
                    ps_q = psq.tile([128, 1024], _dt.float32, tag="fq")
                    ps_k = psk.tile([128, 1024], _dt.float32, tag="fk")
                    for j in range(HB):
                        mm(ps_q[:, 0:R], fwd_st(ma, j, mb), xq[j][:],
                           j == 0, j == HB - 1)
                    for j in range(HB):
                        mm(ps_q[:, R:2 * R], fwd_st(mb_, j, mb), xq[j][:],
                           j == 0, j == HB - 1)
                    q_sb = stp.tile([128, 1024], store, tag="qsb")
                    nc.scalar.mul(q_sb[:], ps_q[:], 1.0)
                    for j in range(HB):
                        mm(ps_k[:, 0:R], fwd_st(ma, j, mb), xk[j][:],
                           j == 0, j == HB - 1)
                    for j in range(HB):
                        mm(ps_k[:, R:2 * R], fwd_st(mb_, j, mb), xk[j][:],
                           j == 0, j == HB - 1)
                    k_sb = stp.tile([128, 1024], store, tag="ksb")
                    nc.scalar.mul(k_sb[:], ps_k[:], 1.0)

                    qre, qim = q_sb[:, 0:R], q_sb[:, R:2 * R]
                    kre, kim = k_sb[:, 0:R], k_sb[:, R:2 * R]
                    t1 = tp.tile([128, R], store, tag="t1")
                    t2 = tp.tile([128, R], store, tag="t2")
                    nc.vector.tensor_mul(t1[:], qre, kre)
                    nc.vector.tensor_mul(t2[:], qim, kim)
                    pre = pp.tile([128, R], store, tag="pre")
                    nc.vector.scalar_tensor_tensor(
                        pre[:], t1[:], 1.0, t2[:],
                        op0=mybir.AluOpType.mult, op1=mybir.AluOpType.add,
                        accum_out=acc[:, gi:gi + 1])
                    t3 = tp.tile([128, R], store, tag="t3")
                    t4 = tp.tile([128, R], store, tag="t4")
                    nc.vector.tensor_mul(t3[:], qim, kre)
                    nc.vector.tensor_mul(t4[:], qre, kim)
                    pim = pp.tile([128, R], store, tag="pim")
                    nc.vector.scalar_tensor_tensor(
                        pim[:], t3[:], 1.0, t4[:],
                        op0=mybir.AluOpType.mult,
                        op1=mybir.AluOpType.subtract,
                        accum_out=acc[:, 4 + gi:5 + gi])
                    if gi == 0:
                        # slot 0 packs DC (re) / Nyquist (im): overwrite with
                        # pure products and patch the two accum elements
                        nc.vector.tensor_copy(pre[0:1, :], t1[0:1, :])
                        nc.vector.tensor_copy(pim[0:1, :], t2[0:1, :])
                        nc.vector.tensor_reduce(
                            acc[0:1, 0:1], t1[0:1, :],
                            axis=mybir.AxisListType.X, op=mybir.AluOpType.add)
                        nc.vector.tensor_reduce(
                            acc[0:1, 4:5], t2[0:1, :],
                            axis=mybir.AxisListType.X, op=mybir.AluOpType.add)
                    pre_sb.append(pre)
                    pim_sb.append(pim)
                state[n] = (pre_sb, pim_sb, acc)

            def inverse(n):
                pre_sb, pim_sb, acc = state[n]
                cm = op.tile([128, LB, R], store, tag="cm")
                for tb in range(HB):
                    ps_uw = psi.tile([128, 1024], _dt.float32, tag="inv")
                    u, w = ps_uw[:, 0:R], ps_uw[:, R:2 * R]
                    mm(u, inv_st(0, 0, tb), pre_sb[0][:], True, False)
                    mm(u, inv_st(0, 1, tb), pre_sb[1][:], False, False)
                    mm(u, inv_st(1, 0, tb), pim_sb[0][:], False, False)
                    mm(u, inv_st(1, 1, tb), pim_sb[1][:], False, True)
                    mm(w, inv_st(2, 0, tb), pre_sb[2][:], True, False)
                    mm(w, inv_st(2, 1, tb), pre_sb[3][:], False, False)
                    mm(w, inv_st(3, 0, tb), pim_sb[2][:], False, False)
                    mm(w, inv_st(3, 1, tb), pim_sb[3][:], False, True)
                    uw_sb = uwp.tile([128, 1024], store, tag="uwsb")
                    nc.scalar.mul(uw_sb[:], ps_uw[:], 1.0)
                    nc.vector.tensor_add(cm[:, tb, :], uw_sb[:, 0:R],
                                         uw_sb[:, R:2 * R])
                    nc.vector.tensor_sub(cm[:, tb + HB, :], uw_sb[:, 0:R],
                                         uw_sb[:, R:2 * R])
                nc.sync.dma_start(
                    corr_d[n].rearrange("(j p) r -> p j r", p=128), cm[:])
                nc.gpsimd.dma_start(pacc_d[n][:], acc[:])

            # software pipeline: fwd(0), fwd(1), inv(0), fwd(2), inv(1), ...
            forward(0)
            for n in range(1, NLOC):
                if n + 1 < NLOC:
                    t = qk.tile([128, LB, R], store, tag="q")
                    nc.sync.dma_start(
                        t[:], q_d[n + 1].rearrange("(j p) r -> p j r", p=128))
                    qm_all.append(t)
                    t = qk.tile([128, LB, R], store, tag="k")
                    nc.scalar.dma_start(
                        t[:], k_d[n + 1].rearrange("(j p) r -> p j r", p=128))
                    km_all.append(t)
                forward(n)
                inverse(n - 1)
            inverse(NLOC - 1)
    nc.compile()
    return nc


def _build_phase2(entries):
    """entries: per output block b, list of (src_block, seg_idx); seg_idx
    indexes the g stationaries tensor (NLOC, 128, nseg*128)."""
    nseg = max(si for segs in entries for _, si in segs) + 1
    nc = bacc.Bacc("TRN2", target_bir_lowering=False, debug=False,
                   num_devices=NCORES)
    v_d = nc.dram_tensor("v", [NLOC, L, R], _dt.bfloat16,
                         kind="ExternalInput").ap()
    # g is host-packed as (NLOC, 128, nseg*128): one contiguous DMA per n;
    # stationary si is the [:, si*128:(si+1)*128] slice.
    g_d = nc.dram_tensor("g", [NLOC, 128, nseg * 128], _dt.bfloat16,
                         kind="ExternalInput").ap()
    out_d = nc.dram_tensor("out", [NLOC, L, R], _dt.bfloat16,
                           kind="ExternalOutput").ap()

    with tile.TileContext(nc) as tc:
        with tc.tile_pool(name="v", bufs=3) as vp, \
             tc.tile_pool(name="g", bufs=NLOC) as gp, \
             tc.tile_pool(name="o", bufs=2) as op, \
             tc.tile_pool(name="ps", bufs=8, space="PSUM") as psp:
            # v[0] split over two queues (first matmul dep), g tiny
            g_sb, v_sb = [], []
            t = vp.tile([128, LB, R], _dt.bfloat16, tag="v")
            nc.sync.dma_start(t[:, 0:HB, :],
                              v_d[0, 0:512].rearrange("(j p) r -> p j r",
                                                      p=128))
            nc.scalar.dma_start(t[:, HB:LB, :],
                                v_d[0, 512:1024].rearrange(
                                    "(j p) r -> p j r", p=128))
            v_sb.append(t)
            for n in range(NLOC):
                tg = gp.tile([128, nseg * 128], _dt.bfloat16, tag="g")
                nc.gpsimd.dma_start(tg[:], g_d[n][:])
                g_sb.append(tg)
            for n in range(NLOC):
                if n + 1 < NLOC:
                    t = vp.tile([128, LB, R], _dt.bfloat16, tag="v")
                    (nc.sync if n % 2 else nc.scalar).dma_start(
                        t[:], v_d[n + 1].rearrange("(j p) r -> p j r", p=128))
                    v_sb.append(t)
                om = op.tile([128, LB, R], _dt.bfloat16, tag="o")
                for b in range(LB):
                    segs = entries[b]
                    ps = psp.tile([128, R], _dt.float32, tag="ps")
                    for i, (a, si) in enumerate(segs):
                        nc.tensor.matmul(ps[:], g_sb[n][:, ts(si, 128)],
                                         v_sb[n][:, a, :],
                                         start=(i == 0),
                                         stop=(i == len(segs) - 1))
                    if b % 2:
                        nc.scalar.mul(om[:, b, :], ps[:], 1.0)
                    else:
                        nc.vector.tensor_copy(om[:, b, :], ps[:])
                (nc.sync if n % 2 else nc.scalar).dma_start(
                    out_d[n].rearrange("(j p) r -> p j r", p=128), om[:])
    nc.compile()
    return nc


_P1_CACHE = {}


def _phase1_nc():
    if "p1" not in _P1_CACHE:
        _P1_CACHE["p1"] = _build_phase1()
    return _P1_CACHE["p1"]


def _run(nc, in_maps, phase):
    res = run_bass_kernel_spmd(nc, in_maps, core_ids=list(range(NCORES)),
                               trace=TRACE[0])
    if TRACE[0]:
        LAST_EXEC_NS[phase] = res.exec_time_ns
    return res.results


def kernel(queries, keys, values):
    queries = np.ascontiguousarray(np.asarray(queries, dtype=np.float32))
    keys = np.ascontiguousarray(np.asarray(keys, dtype=np.float32))
    values = np.ascontiguousarray(np.asarray(values, dtype=np.float32))

    store_np = ml_dtypes.bfloat16
    C5, S5, Mre, Mim, Au, Bu, Aw, Bw = _dft_mats()
    cf = np.concatenate([C5, S5, Mre, Mim], axis=1)   # [512, 1024]
    ci = np.concatenate([Au, Bu, Aw, Bw], axis=1)     # [256, 2048]
    cf = np.ascontiguousarray(cf.astype(np.float32)).astype(store_np)
    ci = np.ascontiguousarray(ci.astype(np.float32)).astype(store_np)

    q3 = queries.reshape(N, L, R)
    k3 = keys.reshape(N, L, R)
    v3 = values.reshape(N, L, R)

    nc1 = _phase1_nc()
    in_maps = []
    for c in range(NCORES):
        sl = slice(c * NLOC, (c + 1) * NLOC)
        in_maps.append({
            "q": q3[sl].astype(store_np),
            "k": k3[sl].astype(store_np),
            "cf": cf, "ci": ci,
        })
    res1 = _run(nc1, in_maps, 0)

    corr = np.concatenate([r["corr"] for r in res1], axis=0)  # (N, L, R)
    pacc = np.concatenate([r["pacc"] for r in res1], axis=0)  # (N, 128, 8)
    # reconstruct mean over (H,E) from per-group P row-sums (host irfft on
    # a 512-vector per batch item)
    pacc = pacc.astype(np.float64)
    per_ = pacc[:, :, 0:2].transpose(0, 2, 1).reshape(N, 256)   # Pe_re sums
    por_ = pacc[:, :, 2:4].transpose(0, 2, 1).reshape(N, 256)   # Po_re
    pei_ = pacc[:, :, 4:6].transpose(0, 2, 1).reshape(N, 256)   # Pe_im
    poi_ = pacc[:, :, 6:8].transpose(0, 2, 1).reshape(N, 256)   # Po_im
    um = per_ @ Au + pei_ @ Bu
    wm = por_ @ Aw + poi_ @ Bw
    mean = np.concatenate([um + wm, um - wm], axis=1) / R       # (N, L)

    g = mean.mean(axis=0)
    idx = np.argsort(-g, kind="stable")[:TOPK]
    w = mean[:, idx]
    e = np.exp(w - w.max(axis=1, keepdims=True))
    w = (e / e.sum(axis=1, keepdims=True)).astype(np.float32)  # (N, TOPK)

    # phase-2 stationaries: out[b*128+j] += w_k * v[(b*128+j+idx_k) mod L]
    # merged per (b, src_block); matrix content is b-independent, so dedup
    # identical segment sets across b.
    seg_of = {}
    pat = []
    entries = [[] for _ in range(LB)]
    for b in range(LB):
        acc = {}
        for kk in range(TOPK):
            sh = int(idx[kk])
            r = sh % 128
            a = ((b * 128 + sh) // 128) % LB
            acc.setdefault(a, []).append(("d1", r, kk))
            if r > 0:
                acc.setdefault((a + 1) % LB, []).append(("d2", r, kk))
        for a, parts in sorted(acc.items()):
            key = tuple(sorted(parts))
            if key not in seg_of:
                seg_of[key] = len(pat)
                pat.append(parts)
            entries[b].append((a, seg_of[key]))
    nseg = len(pat)
    gmat = np.zeros((NLOC * NCORES, nseg, 128, 128), np.float32)
    jj = np.arange(128)
    for si, parts in enumerate(pat):
        for which, r, kk in parts:
            if which == "d1":
                j = jj[: 128 - r]
                gmat[:, si, j + r, j] += w[:, kk][:, None]
            else:
                j = jj[128 - r:]
                gmat[:, si, j - (128 - r), j] += w[:, kk][:, None]
    # pack (NLOC, nseg, 128, 128) -> (NLOC, 128, nseg*128) for 1-DMA-per-n
    gmat = np.ascontiguousarray(
        gmat.transpose(0, 2, 1, 3).reshape(NLOC * NCORES, 128, nseg * 128)
    ).astype(ml_dtypes.bfloat16)

    nc2 = _build_phase2(entries)
    in_maps2 = []
    for c in range(NCORES):
        sl = slice(c * NLOC, (c + 1) * NLOC)
        in_maps2.append({
            "v": v3[sl].astype(ml_dtypes.bfloat16),
            "g": gmat[sl],
        })
    res2 = _run(nc2, in_maps2, 1)
    out = np.concatenate([np.asarray(r["out"], dtype=np.float32)
                          for r in res2], axis=0)             # (N, L, R)

    out_full = out.reshape(N, L, H, E).astype(np.float32)
    corr_full = corr.reshape(N, L, H, E).astype(np.float32)
    return out_full, corr_full


# revision 8
# speedup vs baseline: 1.0510x; 1.0173x over previous
"""Trainium2 Bass kernel for nn_AutoCorrelation (8 NeuronCores, data-parallel over batch).

Algorithm (reference: AutoCorrelation block):
  corr = irfft(rfft(q, L) * conj(rfft(k, L)))        # circular cross-correlation
  top-6 delays from batch-mean of corr (mean over H,E then N)
  out  = sum_k softmax(mean[:, idx])_k * roll(v, -idx_k)

Implementation (two launches, host does only the tiny (N,L) topk/softmax glue):
  - Phase 1: radix-2-real DFT as dense TensorE matmuls, PE-paced (96 matmuls
    per batch item, ~216ns each back-to-back). DVE does the radix-2
    butterflies, complex products and final u+/-w combines (all bf16 2x-mode
    tensor_tensor); ACT stages every PSUM->SBUF bf16 conversion and its
    accumulator collects sum_r u / sum_r w per tau, which IS the (N,L) topk
    mean statistic (host just forms (su+sw)/R, (su-sw)/R). GpSimd does no
    compute (concurrent DVE+Pool execution inflates both ~3x on HW). Inputs
    stream as paired-block DMAs spread over the 3 DMA queues, inverse-const
    load deferred so it can't gate the first matmul.
  - Phase 2: out = sum_k w*roll(v) as PSUM-accumulated matmuls with w-scaled
    shifted-identity stationaries; delay-0-style taps (remainder 0) fold into
    the PSUM->SBUF copy as a DVE scalar_tensor_tensor with a per-partition
    weight vector, removing 8 matmuls per batch item.
"""
import math
import sys

sys.path.insert(0, "/opt/trn_rl_repo")

import numpy as np
import ml_dtypes

import concourse.bass as bass
import concourse.tile as tile
from concourse import bacc, mybir
from concourse.bass import ts
from concourse.bass_utils import run_bass_kernel_spmd

_dt = mybir.dt

N, L, H, E = 32, 1024, 8, 64
R = H * E                 # 512 rows (h,e) per batch item
NCORES = 8
NLOC = N // NCORES        # 4 batch items per core
F2 = 256                  # freqs per radix-2 half (even / odd)
TOPK = int(1.0 * math.log(L))  # 6
LB = L // 128             # 8 l/tau blocks
HB = 4                    # 128-blocks per 512-half

TRACE = [False]           # test.py flips this to collect exec_time_ns
LAST_EXEC_NS = [0, 0]     # phase1, phase2 exec time (when TRACE)


def _dft_mats():
    """Radix-2 split matrices. Forward (contract over l' = 0..511):
    even freqs X[2m] = (x1+x2) @ [C5 | S5m] (S5m slot 0 = f=512 Nyquist),
    odd freqs X[2m+1] = (x1-x2) @ [Mre | Mim] (twiddle folded in).
    Inverse: u = Pe_re@Au + Pe_im@Bu, w = Po_re@Aw + Po_im@Bw,
    corr[t] = u+w, corr[t+512] = u-w."""
    l = np.arange(512)[:, None].astype(np.float64)
    m = np.arange(F2)[None, :].astype(np.float64)
    C5 = np.cos(2 * np.pi * l * m / 512)
    S5 = -np.sin(2 * np.pi * l * m / 512)
    S5[:, 0] = (-1.0) ** np.arange(512)
    Mre = np.cos(2 * np.pi * l * (2 * m + 1) / L)
    Mim = -np.sin(2 * np.pi * l * (2 * m + 1) / L)
    t = np.arange(512)[None, :].astype(np.float64)
    mm = np.arange(F2)[:, None].astype(np.float64)
    Au = (2.0 / L) * np.cos(2 * np.pi * mm * t / 512)
    Bu = -(2.0 / L) * np.sin(2 * np.pi * mm * t / 512)
    Au[0, :] = 1.0 / L
    Bu[0, :] = (1.0 / L) * ((-1.0) ** np.arange(512))
    Aw = (2.0 / L) * np.cos(2 * np.pi * t * (2 * mm + 1) / L)
    Bw = -(2.0 / L) * np.sin(2 * np.pi * t * (2 * mm + 1) / L)
    return C5, S5, Mre, Mim, Au, Bu, Aw, Bw


def _build_phase1():
    store = _dt.bfloat16

    nc = bacc.Bacc("TRN2", target_bir_lowering=False, debug=False,
                   num_devices=NCORES)
    q_d = nc.dram_tensor("q", [NLOC, L, R], store, kind="ExternalInput").ap()
    k_d = nc.dram_tensor("k", [NLOC, L, R], store, kind="ExternalInput").ap()
    # cf cols: c5 | s5 | mre | mim  (each [512, 256])
    cf_d = nc.dram_tensor("cf", [512, 4 * F2], store,
                          kind="ExternalInput").ap()
    # ci cols: au | bu | aw | bw  (each [256, 512])
    ci_d = nc.dram_tensor("ci", [F2, 4 * 512], store,
                          kind="ExternalInput").ap()
    corr_d = nc.dram_tensor("corr", [NLOC, L, R], store,
                            kind="ExternalOutput").ap()
    # stat: cols tb = sum_r u[tb*128+p, r], cols 4+tb = sum_r w[...]
    pacc_d = nc.dram_tensor("pacc", [NLOC, 128, 8], _dt.float32,
                            kind="ExternalOutput").ap()

    def mm(ps, lhsT, rhs, start, stop):
        nc.tensor.matmul(ps, lhsT, rhs, start=start, stop=stop)

    def rearr(ap):
        return ap.rearrange("(j p) r -> p j r", p=128)

    with tile.TileContext(nc) as tc:
        with tc.tile_pool(name="const", bufs=1) as cp, \
             tc.tile_pool(name="qk", bufs=2) as qk, \
             tc.tile_pool(name="ed", bufs=8) as edp, \
             tc.tile_pool(name="st", bufs=5) as stp, \
             tc.tile_pool(name="tmp", bufs=3) as tp, \
             tc.tile_pool(name="pp", bufs=10) as pp, \
             tc.tile_pool(name="uw", bufs=6) as uwp, \
             tc.tile_pool(name="out", bufs=2) as op, \
             tc.tile_pool(name="psq", bufs=1, space="PSUM") as psq, \
             tc.tile_pool(name="psk", bufs=1, space="PSUM") as psk, \
             tc.tile_pool(name="psi", bufs=2, space="PSUM") as psi:

            # ---- pipeline fill: 3 dma queues, need-ordered ----
            # sync:   cf_a, q0{0,4}, q0{2,6}, k1, q3 (+corr outs n=0,2)
            # scalar: cf_b, q0{1,5}, q0{3,7}, q2, k3 (+corr outs n=1,3)
            # gpsimd: k0 pairs x4, q1, ci, k2        (+pacc outs)
            cf_sb = cp.tile([128, HB, 4 * F2], store, tag="cf")
            nc.sync.dma_start(
                cf_sb[:, :, 0:2 * F2],
                cf_d[:, 0:2 * F2].rearrange("(j p) c -> p j c", p=128))
            nc.scalar.dma_start(
                cf_sb[:, :, 2 * F2:4 * F2],
                cf_d[:, 2 * F2:4 * F2].rearrange("(j p) c -> p j c", p=128))

            qm_all, km_all = [None] * NLOC, [None] * NLOC
            t = qk.tile([128, LB, R], store, tag="q")
            qr = rearr(q_d[0])
            nc.sync.dma_start(t[:, 0:5:4, :], qr[:, 0:5:4, :])
            nc.scalar.dma_start(t[:, 1:6:4, :], qr[:, 1:6:4, :])
            nc.sync.dma_start(t[:, 2:7:4, :], qr[:, 2:7:4, :])
            nc.scalar.dma_start(t[:, 3:8:4, :], qr[:, 3:8:4, :])
            qm_all[0] = t
            t = qk.tile([128, LB, R], store, tag="k")
            kr = rearr(k_d[0])
            for j in range(HB):
                nc.gpsimd.dma_start(t[:, j:j + 5:4, :], kr[:, j:j + 5:4, :])
            km_all[0] = t

            t = qk.tile([128, LB, R], store, tag="q")
            nc.gpsimd.dma_start(t[:], rearr(q_d[1]))
            qm_all[1] = t
            t = qk.tile([128, LB, R], store, tag="k")
            nc.sync.dma_start(t[:], rearr(k_d[1]))
            km_all[1] = t

            ci_sb = cp.tile([128, 2, 4 * 512], store, tag="ci")
            nc.gpsimd.dma_start(ci_sb[:],
                                ci_d.rearrange("(g p) c -> p g c", p=128))
            if NLOC > 2:
                t = qk.tile([128, LB, R], store, tag="q")
                nc.scalar.dma_start(t[:], rearr(q_d[2]))
                qm_all[2] = t
                t = qk.tile([128, LB, R], store, tag="k")
                nc.gpsimd.dma_start(t[:], rearr(k_d[2]))
                km_all[2] = t
            if NLOC > 3:
                t = qk.tile([128, LB, R], store, tag="q")
                nc.sync.dma_start(t[:], rearr(q_d[3]))
                qm_all[3] = t
                t = qk.tile([128, LB, R], store, tag="k")
                nc.scalar.dma_start(t[:], rearr(k_d[3]))
                km_all[3] = t

            def fwd_st(name_i, j, mb):
                off = name_i * F2 + mb * 128
                return cf_sb[:, j, off:off + 128]

            def inv_st(name_i, gb, tb):
                off = name_i * 512 + tb * 128
                return ci_sb[:, gb, off:off + 128]

            state = [None] * NLOC  # per-n (pre_sb, pim_sb) for inverse
            accs = [None] * NLOC

            def forward(n):
                qm, km = qm_all[n], km_all[n]
                # radix-2 butterflies: all on DVE (full-width bf16 2x ops)
                eq, dq, ek, dk = [], [], [], []
                for tag, lst, x, fn in (("eq", eq, qm, "tensor_add"),
                                        ("dq", dq, qm, "tensor_sub"),
                                        ("ek", ek, km, "tensor_add"),
                                        ("dk", dk, km, "tensor_sub")):
                    for j in range(HB):
                        t = edp.tile([128, R], store, tag=tag)
                        getattr(nc.vector, fn)(t[:], x[:, j, :],
                                               x[:, j + 4, :])
                        lst.append(t)

                pre_sb, pim_sb = [], []
                groups = [(0, 1, eq, ek, 0), (0, 1, eq, ek, 1),
                          (2, 3, dq, dk, 0), (2, 3, dq, dk, 1)]
                for gi, (ma, mb_, xq, xk, mb) in enumerate(groups):
                    ps_q = psq.tile([128, 1024], _dt.float32, tag="fq")
                    ps_k = psk.tile([128, 1024], _dt.float32, tag="fk")
                    for j in range(HB):
                        mm(ps_q[:, 0:R], fwd_st(ma, j, mb), xq[j][:],
                           j == 0, j == HB - 1)
                    for j in range(HB):
                        mm(ps_q[:, R:2 * R], fwd_st(mb_, j, mb), xq[j][:],
                           j == 0, j == HB - 1)
                    q_sb = stp.tile([128, 1024], store, tag="qsb")
                    nc.scalar.mul(q_sb[:], ps_q[:], 1.0)
                    for j in range(HB):
                        mm(ps_k[:, 0:R], fwd_st(ma, j, mb), xk[j][:],
                           j == 0, j == HB - 1)
                    for j in range(HB):
                        mm(ps_k[:, R:2 * R], fwd_st(mb_, j, mb), xk[j][:],
                           j == 0, j == HB - 1)
                    k_sb = stp.tile([128, 1024], store, tag="ksb")
                    nc.scalar.mul(k_sb[:], ps_k[:], 1.0)

                    qre, qim = q_sb[:, 0:R], q_sb[:, R:2 * R]
                    kre, kim = k_sb[:, 0:R], k_sb[:, R:2 * R]
                    t1 = tp.tile([128, R], store, tag="t1")
                    t2 = tp.tile([128, R], store, tag="t2")
                    nc.vector.tensor_mul(t1[:], qre, kre)
                    nc.vector.tensor_mul(t2[:], qim, kim)
                    pre = pp.tile([128, R], store, tag="pre")
                    nc.vector.tensor_add(pre[:], t1[:], t2[:])
                    t3 = tp.tile([128, R], store, tag="t3")
                    t4 = tp.tile([128, R], store, tag="t4")
                    nc.vector.tensor_mul(t3[:], qim, kre)
                    nc.vector.tensor_mul(t4[:], qre, kim)
                    pim = pp.tile([128, R], store, tag="pim")
                    nc.vector.tensor_sub(pim[:], t3[:], t4[:])
                    if gi == 0:
                        # slot 0 packs DC (re) / Nyquist (im): overwrite
                        # with the pure products
                        nc.vector.tensor_copy(pre[0:1, :], t1[0:1, :])
                        nc.vector.tensor_copy(pim[0:1, :], t2[0:1, :])
                    pre_sb.append(pre)
                    pim_sb.append(pim)
                state[n] = (pre_sb, pim_sb)

            def inverse(n):
                pre_sb, pim_sb = state[n]
                acc = op.tile([128, 8], _dt.float32, tag="acc")
                accs[n] = acc
                cm = op.tile([128, LB, R], store, tag="cm")
                for tb in range(HB):
                    ps_uw = psi.tile([128, 1024], _dt.float32, tag="inv")
                    u, w = ps_uw[:, 0:R], ps_uw[:, R:2 * R]
                    mm(u, inv_st(0, 0, tb), pre_sb[0][:], True, False)
                    mm(u, inv_st(0, 1, tb), pre_sb[1][:], False, False)
                    mm(u, inv_st(1, 0, tb), pim_sb[0][:], False, False)
                    mm(u, inv_st(1, 1, tb), pim_sb[1][:], False, True)
                    mm(w, inv_st(2, 0, tb), pre_sb[2][:], True, False)
                    mm(w, inv_st(2, 1, tb), pre_sb[3][:], False, False)
                    mm(w, inv_st(3, 0, tb), pim_sb[2][:], False, False)
                    mm(w, inv_st(3, 1, tb), pim_sb[3][:], False, True)
                    uw_sb = uwp.tile([128, 1024], store, tag="uwsb")
                    # staging copies double as the topk-stat reduction:
                    # accumulator = sum_r u / sum_r w per tau row
                    nc.scalar.activation(
                        uw_sb[:, 0:R], u, mybir.ActivationFunctionType.Copy,
                        bias=0.0, scale=1.0, accum_out=acc[:, tb:tb + 1])
                    nc.scalar.activation(
                        uw_sb[:, R:2 * R], w,
                        mybir.ActivationFunctionType.Copy,
                        bias=0.0, scale=1.0,
                        accum_out=acc[:, 4 + tb:5 + tb])
                    nc.vector.tensor_add(cm[:, tb, :], uw_sb[:, 0:R],
                                         uw_sb[:, R:2 * R])
                    nc.vector.tensor_sub(cm[:, tb + HB, :], uw_sb[:, 0:R],
                                         uw_sb[:, R:2 * R])
                    if tb == 1:
                        eng = nc.sync if n % 2 == 0 else nc.scalar
                        eng.dma_start(rearr(corr_d[n])[:, 0:2, :],
                                      cm[:, 0:2, :])
                        eng.dma_start(rearr(corr_d[n])[:, 4:6, :],
                                      cm[:, 4:6, :])
                eng = nc.sync if n % 2 == 0 else nc.scalar
                eng.dma_start(rearr(corr_d[n])[:, 2:4, :], cm[:, 2:4, :])
                eng.dma_start(rearr(corr_d[n])[:, 6:8, :], cm[:, 6:8, :])
                nc.gpsimd.dma_start(pacc_d[n][:], acc[:])

            # software pipeline: fwd(0), fwd(1), inv(0), fwd(2), inv(1), ...
            forward(0)
            for n in range(1, NLOC):
                forward(n)
                inverse(n - 1)
            inverse(NLOC - 1)
    nc.compile()
    return nc


def _build_phase2(entries, fuse_hi):
    """entries: per output block b, list of (src_block, seg_idx); seg_idx
    indexes the g stationaries tensor (NLOC, 128, nseg*128). fuse_hi: the
    block offset of the remainder-0 tap folded into the out-copy STT (its
    per-partition weight comes from the wv input; 0/w=0 when unused)."""
    nseg = max((si for segs in entries for _, si in segs), default=-1) + 1
    nseg = max(nseg, 1)
    nc = bacc.Bacc("TRN2", target_bir_lowering=False, debug=False,
                   num_devices=NCORES)
    v_d = nc.dram_tensor("v", [NLOC, L, R], _dt.bfloat16,
                         kind="ExternalInput").ap()
    g_d = nc.dram_tensor("g", [NLOC, 128, nseg * 128], _dt.bfloat16,
                         kind="ExternalInput").ap()
    wv_d = nc.dram_tensor("wv", [NLOC, 128, 1], _dt.float32,
                          kind="ExternalInput").ap()
    out_d = nc.dram_tensor("out", [NLOC, L, R], _dt.bfloat16,
                           kind="ExternalOutput").ap()

    def rearr(ap):
        return ap.rearrange("(j p) r -> p j r", p=128)

    with tile.TileContext(nc) as tc:
        with tc.tile_pool(name="v", bufs=3) as vp, \
             tc.tile_pool(name="g", bufs=NLOC) as gp, \
             tc.tile_pool(name="o", bufs=2) as op, \
             tc.tile_pool(name="ps", bufs=8, space="PSUM") as psp:
            g_sb, wv_sb, v_sb = [], [], []
            t = vp.tile([128, LB, R], _dt.bfloat16, tag="v")
            vr = rearr(v_d[0])
            nc.sync.dma_start(t[:, 0:5:4, :], vr[:, 0:5:4, :])
            nc.scalar.dma_start(t[:, 1:6:4, :], vr[:, 1:6:4, :])
            nc.sync.dma_start(t[:, 2:7:4, :], vr[:, 2:7:4, :])
            nc.scalar.dma_start(t[:, 3:8:4, :], vr[:, 3:8:4, :])
            v_sb.append(t)
            for n in range(NLOC):
                tg = gp.tile([128, nseg * 128], _dt.bfloat16, tag="g")
                nc.gpsimd.dma_start(tg[:], g_d[n][:])
                g_sb.append(tg)
                tw = gp.tile([128, 1], _dt.float32, tag="wv")
                nc.gpsimd.dma_start(tw[:], wv_d[n][:])
                wv_sb.append(tw)
            for n in range(NLOC):
                if n + 1 < NLOC:
                    t = vp.tile([128, LB, R], _dt.bfloat16, tag="v")
                    (nc.sync if n % 2 else nc.scalar).dma_start(
                        t[:], rearr(v_d[n + 1]))
                    v_sb.append(t)
                om = op.tile([128, LB, R], _dt.bfloat16, tag="o")
                for b in range(LB):
                    segs = entries[b]
                    ps = psp.tile([128, R], _dt.float32, tag="ps")
                    for i, (a, si) in enumerate(segs):
                        nc.tensor.matmul(ps[:], g_sb[n][:, ts(si, 128)],
                                         v_sb[n][:, a, :],
                                         start=(i == 0),
                                         stop=(i == len(segs) - 1))
                    # fused remainder-0 tap: om = v[b+hi]*wv + psum
                    nc.vector.scalar_tensor_tensor(
                        om[:, b, :], v_sb[n][:, (b + fuse_hi) % LB, :],
                        wv_sb[n][:, 0:1], ps[:],
                        op0=mybir.AluOpType.mult, op1=mybir.AluOpType.add)
                    if b == 3:
                        (nc.sync if n % 2 else nc.scalar).dma_start(
                            rearr(out_d[n])[:, 0:4, :], om[:, 0:4, :])
                (nc.sync if n % 2 else nc.scalar).dma_start(
                    rearr(out_d[n])[:, 4:8, :], om[:, 4:8, :])
    nc.compile()
    return nc


_P1_CACHE = {}


def _phase1_nc():
    if "p1" not in _P1_CACHE:
        _P1_CACHE["p1"] = _build_phase1()
    return _P1_CACHE["p1"]


def _run(nc, in_maps, phase):
    res = run_bass_kernel_spmd(nc, in_maps, core_ids=list(range(NCORES)),
                               trace=TRACE[0])
    if TRACE[0]:
        LAST_EXEC_NS[phase] = res.exec_time_ns
    return res.results


def kernel(queries, keys, values):
    queries = np.ascontiguousarray(np.asarray(queries, dtype=np.float32))
    keys = np.ascontiguousarray(np.asarray(keys, dtype=np.float32))
    values = np.ascontiguousarray(np.asarray(values, dtype=np.float32))

    store_np = ml_dtypes.bfloat16
    C5, S5, Mre, Mim, Au, Bu, Aw, Bw = _dft_mats()
    cf = np.concatenate([C5, S5, Mre, Mim], axis=1)   # [512, 1024]
    ci = np.concatenate([Au, Bu, Aw, Bw], axis=1)     # [256, 2048]
    cf = np.ascontiguousarray(cf.astype(np.float32)).astype(store_np)
    ci = np.ascontiguousarray(ci.astype(np.float32)).astype(store_np)

    q3 = queries.reshape(N, L, R)
    k3 = keys.reshape(N, L, R)
    v3 = values.reshape(N, L, R)

    nc1 = _phase1_nc()
    in_maps = []
    for c in range(NCORES):
        sl = slice(c * NLOC, (c + 1) * NLOC)
        in_maps.append({
            "q": q3[sl].astype(store_np),
            "k": k3[sl].astype(store_np),
            "cf": cf, "ci": ci,
        })
    res1 = _run(nc1, in_maps, 0)

    corr = np.concatenate([r["corr"] for r in res1], axis=0)  # (N, L, R)
    pacc = np.concatenate([r["pacc"] for r in res1], axis=0)  # (N, 128, 8)
    # stat: mean[n, tb*128+p] = (su+sw)/R, mean[n, 512+tb*128+p] = (su-sw)/R
    pacc = pacc.astype(np.float64)
    su = pacc[:, :, 0:4].transpose(0, 2, 1).reshape(N, 512)
    sw = pacc[:, :, 4:8].transpose(0, 2, 1).reshape(N, 512)
    mean = np.concatenate([su + sw, su - sw], axis=1) / R       # (N, L)

    g = mean.mean(axis=0)
    idx = np.argsort(-g, kind="stable")[:TOPK]
    w = mean[:, idx]
    e = np.exp(w - w.max(axis=1, keepdims=True))
    w = (e / e.sum(axis=1, keepdims=True)).astype(np.float32)  # (N, TOPK)

    # phase-2 stationaries: out[b*128+j] += w_k * v[(b*128+j+idx_k) mod L].
    # One remainder-0 tap is folded into the out-copy STT (fuse_hi / wv);
    # the rest are merged per (b, src_block) into banded stationaries,
    # deduped across b (matrix content is b-independent).
    fuse_k = next((kk for kk in range(TOPK) if int(idx[kk]) % 128 == 0),
                  None)
    fuse_hi = (int(idx[fuse_k]) // 128) % LB if fuse_k is not None else 0
    wv = (w[:, fuse_k] if fuse_k is not None
          else np.zeros(N, np.float32))                        # (N,)
    wv = np.ascontiguousarray(
        np.broadcast_to(wv[:, None, None], (N, 128, 1))).astype(np.float32)

    seg_of = {}
    pat = []
    entries = [[] for _ in range(LB)]
    for b in range(LB):
        acc = {}
        for kk in range(TOPK):
            if kk == fuse_k:
                continue
            sh = int(idx[kk])
            r = sh % 128
            a = ((b * 128 + sh) // 128) % LB
            acc.setdefault(a, []).append(("d1", r, kk))
            if r > 0:
                acc.setdefault((a + 1) % LB, []).append(("d2", r, kk))
        for a, parts in sorted(acc.items()):
            key = tuple(sorted(parts))
            if key not in seg_of:
                seg_of[key] = len(pat)
                pat.append(parts)
            entries[b].append((a, seg_of[key]))
    nseg = max(len(pat), 1)
    gmat = np.zeros((NLOC * NCORES, nseg, 128, 128), np.float32)
    jj = np.arange(128)
    for si, parts in enumerate(pat):
        for which, r, kk in parts:
            if which == "d1":
                j = jj[: 128 - r]
                gmat[:, si, j + r, j] += w[:, kk][:, None]
            else:
                j = jj[128 - r:]
                gmat[:, si, j - (128 - r), j] += w[:, kk][:, None]
    # pack (NLOC, nseg, 128, 128) -> (NLOC, 128, nseg*128) for 1-DMA-per-n
    gmat = np.ascontiguousarray(
        gmat.transpose(0, 2, 1, 3).reshape(NLOC * NCORES, 128, nseg * 128)
    ).astype(ml_dtypes.bfloat16)

    nc2 = _build_phase2(entries, fuse_hi)
    in_maps2 = []
    for c in range(NCORES):
        sl = slice(c * NLOC, (c + 1) * NLOC)
        in_maps2.append({
            "v": v3[sl].astype(ml_dtypes.bfloat16),
            "g": gmat[sl],
            "wv": wv[sl],
        })
    res2 = _run(nc2, in_maps2, 1)
    out = np.concatenate([np.asarray(r["out"], dtype=np.float32)
                          for r in res2], axis=0)             # (N, L, R)

    out_full = out.reshape(N, L, H, E).astype(np.float32)
    corr_full = corr.reshape(N, L, H, E).astype(np.float32)
    return out_full, corr_full
